# revision 10
# baseline (speedup 1.0000x reference)
"""GAT (2-layer, PyG-style) on 8 Trainium2 NeuronCores via Bass/Tile.

Design (v4 — identity-slot layout, gather-free layer 1):
  * Nodes are sorted by in-degree and assigned round-robin to (core, slot)
    positions; each dst node owns one partition row of one block, and its
    incoming edges occupy k-slots along the free dim (k=0 is the self loop).
    Segment softmax/sum therefore needs no one-hot matmuls.
  * Layer 1 never gathers: the host expands x into per-core edge-order
    operand xE, and h(+es,ed) is computed per edge as xE @ W1ext directly
    (2.2x redundant FLOPs but sequential DMA and zero SWDGE cost).
    Padding edge slots use a host-built x_neg vector with x_neg@Wa1 = -200
    per head, so exp(lrelu(es+ed)) underflows to exactly 0.
  * All feature data is fp16 (fp32 accumulation in PSUM); h is stored
    head-minor ("c-major", col = c*8+h) so per-head broadcasts have a packed
    last dim (DVE 2x mode).
  * Layer-2 features (64 h2 + duplicated es2/ed2 = 68 fp16 cols) are
    AllGathered, then gathered per edge with [128,1] indirect DMAs.
"""
import os
import sys

sys.path.insert(0, "/opt/trn_rl_repo")

import numpy as np

import concourse.bass as bass
import concourse.mybir as mybir
import concourse.tile as tile
from concourse import bacc, bass_utils, library_config

P = 128
NCORES = 8
NEG_SLOPE = 0.2
R1 = 272          # hxE row: 256 h (c-major) + 8 es + 8 ed
R2 = 68           # hx2 row: 64 h2 + es2,es2 + ed2,ed2
NEG_ES = -200.0   # padding es -> exp(lrelu(es+ed)) underflows to 0


def plan(src, dst, n_nodes, n_cores=NCORES):
    """Host-side graph partitioning / layout planning."""
    deg = np.bincount(dst, minlength=n_nodes).astype(np.int64) + 1  # + self loop
    order = np.argsort(deg, kind="stable")          # node ids, ascending degree
    nblk = (n_nodes + n_cores * P - 1) // (n_cores * P)
    npc = nblk * P
    npos = npc * n_cores
    row_neg = npos

    ranks = np.arange(n_nodes)
    pos_of_node = np.empty(n_nodes, np.int64)
    pos_of_node[order] = (ranks % n_cores) * npc + ranks // n_cores

    deg_by_rank = np.zeros(npos, np.int64)
    deg_by_rank[:n_nodes] = deg[order]
    Kb = deg_by_rank.reshape(nblk, n_cores * P).max(1)
    Kb = np.maximum(Kb, 1)
    tob = np.concatenate([[0], np.cumsum(Kb)]).astype(np.int64)
    T = int(tob[-1])

    # srcnode[c, p, t]: node id feeding edge-slot (p, b, k) of core c;
    # -1 = padding (x_neg), self loop at k=0 (dummy positions get -2 -> x=0)
    srcnode = np.full((n_cores, P, T), -1, np.int64)
    idx = np.full((n_cores, P, T), row_neg, np.int32)

    # k = 0: self loop
    node_by_pos = np.full(npos, -2, np.int64)
    node_by_pos[pos_of_node] = np.arange(n_nodes)
    own = node_by_pos.reshape(n_cores, nblk, P).transpose(0, 2, 1)  # [C,P,nblk]
    srcnode[:, :, tob[:-1]] = own
    pos_by_pos = np.arange(npos)
    ownpos = pos_by_pos.reshape(n_cores, nblk, P).transpose(0, 2, 1)
    idx[:, :, tob[:-1]] = ownpos

    dpos = pos_of_node[dst]
    spos = pos_of_node[src].astype(np.int32)
    order_e = np.argsort(dpos, kind="stable")
    dpos_s = dpos[order_e]
    src_s = src[order_e]
    spos_s = spos[order_e]
    counts = np.bincount(dpos_s, minlength=npos)
    starts = np.concatenate([[0], np.cumsum(counts)])[:-1]
    occ = np.arange(len(dpos_s)) - starts[dpos_s]
    c = dpos_s // npc
    rem = dpos_s % npc
    b = rem // P
    p = rem % P
    t = tob[b] + occ + 1
    srcnode[c, p, t] = src_s
    idx[c, p, t] = spos_s

    # ---- layer-2 dma_gather tables (lo/hi int16 windows) ----
    # hx2loc row space: [neg0 x128 | pos<SPLIT (rows+128) | neg1 x128 | rest (rows+256)]
    SPLIT = min(32384, npos)           # positions below go to the LO window
    NI = 1024                          # max idxs per dma_gather chunk
    row_of_pos = np.where(np.arange(npos) < SPLIT,
                          np.arange(npos) + P, np.arange(npos) + 2 * P)
    # per (core, p, block): lo and hi edge-row lists (self loop included)
    ownpos_f = ownpos                  # [C, P, nblk] own positions
    srcpos = np.full((n_cores, P, T), -1, np.int64)
    srcpos[:, :, tob[:-1]] = ownpos_f
    srcpos[c, p, t] = spos_s
    K2lo = np.zeros(nblk, np.int64)
    K2hi = np.zeros(nblk, np.int64)
    lo_of = []
    for cc in range(n_cores):
        lo_of.append(None)
    # count per (c,p,b)
    nlo = np.zeros((n_cores, P, nblk), np.int64)
    nhi = np.zeros((n_cores, P, nblk), np.int64)
    for b in range(nblk):
        sl = srcpos[:, :, tob[b]:tob[b + 1]]
        valid = sl >= 0
        lo = valid & (sl < SPLIT)
        nlo[:, :, b] = lo.sum(2)
        nhi[:, :, b] = (valid & ~lo).sum(2)
    K2lo = np.maximum(nlo.max(axis=(0, 1)), 1)
    K2hi = np.maximum(nhi.max(axis=(0, 1)), 1)
    # chunk layout: per block, lo chunks then hi chunks, each <= NI idxs
    chunks = []                        # (b, half, k0, kn, col0) ; col0 into idx16
    col = 0
    for b in range(nblk):
        for half, Kh in ((0, int(K2lo[b])), (1, int(K2hi[b]))):
            k0 = 0
            while k0 < Kh:
                kn = min(NI // P, Kh - k0)
                chunks.append((b, half, k0, kn, col))
                col += kn * P // 16
            # noop
                k0 += kn
    T2c = col
    idx16 = np.zeros((n_cores, P, T2c), np.int16)
    for cc in range(n_cores):
        for b in range(nblk):
            sl = srcpos[cc, :, tob[b]:tob[b + 1]]   # [P, Kb]
            for half in (0, 1):
                Kh = int(K2lo[b]) if half == 0 else int(K2hi[b])
                # per-partition edge rows of this half
                arr = np.zeros((P, Kh), np.int64)   # default negrow idx 0
                for p_ in range(P):
                    v = sl[p_][sl[p_] >= 0]
                    v = v[v < SPLIT] if half == 0 else v[v >= SPLIT]
                    r = row_of_pos[v]
                    r = r if half == 0 else r - SPLIT - P
                    arr[p_, :len(r)] = r
                for (bb, hh, k0, kn, col0) in chunks:
                    if bb != b or hh != half:
                        continue
                    flat = arr[:, k0:k0 + kn].T.reshape(-1)   # k-major (k*128+p)
                    w = flat.reshape(-1, 16).T.astype(np.int16)
                    idx16[cc, :, col0:col0 + kn * P // 16] = np.tile(w, (8, 1))
    ownrow = np.zeros((n_cores, P, nblk), np.int32)
    for cc in range(n_cores):
        ownrow[cc] = row_of_pos[ownpos_f[cc]]
    return dict(npc=npc, nblk=nblk, npos=npos, row_neg=row_neg, Kb=Kb,
                tob=tob, T=T, idx=idx, srcnode=srcnode, order=order,
                pos_of_node=pos_of_node, SPLIT=SPLIT, NI=NI,
                K2lo=K2lo, K2hi=K2hi, chunks=chunks, T2c=T2c, idx16=idx16,
                ownrow=ownrow)


def build(pl, cin, heads, hid, cout):
    HC = heads * hid          # 256
    npc, nblk, npos = pl["npc"], pl["nblk"], pl["npos"]
    Kb, tob, T = pl["Kb"], pl["tob"], pl["T"]
    K2lo, K2hi, chunks, T2c = pl["K2lo"], pl["K2hi"], pl["chunks"], pl["T2c"]
    SPLIT = pl["SPLIT"]
    R2D = 128                 # hx2loc row (padded for 256B dma_gather stride)
    NROW2 = npos + 2 * P      # + neg0/neg1 padding rows
    NBLK_LO = SPLIT // P      # blocks whose rows sit in the LO window
    NT = npos // P
    W2C = 2 * R2

    nc = bacc.Bacc("TRN2")
    f16 = mybir.dt.float16
    f32 = mybir.dt.float32

    xE = nc.dram_tensor("xE", [cin, T * P], f16, kind="ExternalInput")
    Wx1 = nc.dram_tensor("Wx1", [cin, R1], f16, kind="ExternalInput")
    Wx2 = nc.dram_tensor("Wx2", [P, W2C], f16, kind="ExternalInput")
    b1r = nc.dram_tensor("b1r", [P, HC], f16, kind="ExternalInput")
    b2r = nc.dram_tensor("b2r", [P, cout], f32, kind="ExternalInput")
    identD = nc.dram_tensor("identD", [P, P], f16, kind="ExternalInput")
    idx16 = nc.dram_tensor("idx16", [P, T2c], mybir.dt.int16,
                           kind="ExternalInput")
    ownrow = nc.dram_tensor("ownrow", [P, nblk], mybir.dt.int32,
                            kind="ExternalInput")
    out = nc.dram_tensor("out", [npc, cout], f32, kind="ExternalOutput")

    hx2in = nc.dram_tensor("hx2in", [npc, R2], f16)
    hx2 = nc.dram_tensor("hx2", [npos, R2], f16, addr_space="Shared")
    hx2loc = nc.dram_tensor("hx2loc", [NROW2, R2D], f16)

    AF = mybir.ActivationFunctionType
    OP = mybir.AluOpType
    AX = mybir.AxisListType

    with tile.TileContext(nc) as tc:
        with tc.tile_pool(name="const", bufs=1) as cp:
            ident_sb = cp.tile([P, P], f16)
            b1_sb = cp.tile([P, HC], f16)
            b2_sb = cp.tile([P, cout], f32)
            w1_sb = cp.tile([cin, R1], f16)
            w2_sb = cp.tile([P, W2C], f16)
            idx_sb = cp.tile([P, T2c], mybir.dt.int16)
            own_sb = cp.tile([P, nblk], mybir.dt.int32)
            nc.gpsimd.load_library(library_config.mlp)
            nc.sync.dma_start(out=ident_sb[:], in_=identD[:, :])
            nc.sync.dma_start(out=b1_sb[:], in_=b1r[:, :])
            nc.sync.dma_start(out=b2_sb[:], in_=b2r[:, :])
            nc.sync.dma_start(out=w1_sb[:], in_=Wx1[:, :])
            nc.sync.dma_start(out=w2_sb[:], in_=Wx2[:, :])
            nc.sync.dma_start(out=idx_sb[:], in_=idx16[:, :])
            nc.sync.dma_start(out=own_sb[:], in_=ownrow[:, :])

            with tc.tile_pool(name="ps2", bufs=2, space="PSUM") as ps2, \
                 tc.tile_pool(name="ps_acc", bufs=2, space="PSUM") as ps_acc, \
                 tc.tile_pool(name="ps_tr", bufs=1, space="PSUM") as ps_tr, \
                 tc.tile_pool(name="ps_h2", bufs=1, space="PSUM") as ps_h2:

                # ---------------- layer-1: edge-order dense + attention ----------------
                with tc.tile_pool(name="e1", bufs=2) as e1, \
                     tc.tile_pool(name="f1", bufs=2) as f1, \
                     tc.tile_pool(name="x1", bufs=3) as x1:
                    for b in range(nblk):
                        K = int(Kb[b])
                        t0 = int(tob[b])
                        xe = x1.tile([cin, K * P], f16, tag="xe")
                        nc.sync.dma_start(out=xe[:, :],
                                          in_=xE[:, t0 * P : (t0 + K) * P])
                        hxg = e1.tile([P, K, R1], f16, tag="hxg")
                        for j0 in range(0, K, 2):
                            jn = min(2, K - j0)
                            psh = ps2.tile([P, 1024], f32, tag="psh",
                                           space="PSUM")
                            for j in range(jn):
                                nc.tensor.matmul(
                                    out=psh[:, j * 512 : j * 512 + R1],
                                    lhsT=xe[:, (j0 + j) * P : (j0 + j + 1) * P],
                                    rhs=w1_sb[:, :], start=True, stop=True)
                            pview = (psh[:, :].rearrange(
                                "p (j r) -> p j r", r=512)[:, :, :R1]
                                if jn == 2 else psh[:, :R1].unsqueeze(1))
                            if (j0 // 2) % 2 == 0:
                                nc.scalar.copy(out=hxg[:, j0 : j0 + jn, :],
                                               in_=pview)
                            else:
                                nc.vector.tensor_copy(out=hxg[:, j0 : j0 + jn, :],
                                                      in_=pview)
                        # e = es[src] + ed[dst] (dst's own ed from self row k=0)
                        ex = e1.tile([P, K, heads], f16, tag="ex")
                        nc.vector.tensor_tensor(
                            out=ex[:, :, :], in0=hxg[:, :, HC : HC + heads],
                            in1=hxg[:, 0, HC + heads : R1].unsqueeze(1)
                                .broadcast_to([P, K, heads]),
                            op=OP.add)
                        mx = e1.tile([P, K, heads], f16, tag="mx")
                        nc.vector.tensor_scalar_max(mx[:, :, :], ex[:, :, :], 0.0)
                        nc.vector.tensor_scalar_min(ex[:, :, :], ex[:, :, :], 0.0)
                        nc.vector.scalar_tensor_tensor(
                            out=ex[:, :, :], in0=ex[:, :, :], scalar=NEG_SLOPE,
                            in1=mx[:, :, :], op0=OP.mult, op1=OP.add)
                        nc.scalar.activation(ex[:, :, :], ex[:, :, :], AF.Exp)
                        den = f1.tile([P, heads], f32, tag="den")
                        nc.vector.tensor_reduce(
                            out=den[:, :], in_=ex[:, :, :].transpose([0, 2, 1]),
                            axis=AX.X, op=OP.add)
                        nc.vector.tensor_tensor(
                            out=hxg[:, :, :HC].rearrange(
                                "p k (c h) -> p k c h", h=heads),
                            in0=hxg[:, :, :HC].rearrange(
                                "p k (c h) -> p k c h", h=heads),
                            in1=ex[:, :, :].unsqueeze(2)
                                .broadcast_to([P, K, hid, heads]),
                            op=OP.mult)
                        acc = ps_acc.tile([P, HC], f32, tag="acc", space="PSUM")
                        for k in range(K):
                            nc.tensor.matmul(out=acc[:, :], lhsT=ident_sb[:],
                                             rhs=hxg[:, k, :HC],
                                             start=(k == 0), stop=(k == K - 1))
                        # ---- flush: normalize, +b1, ELU, @W2ext, store ----
                        denr = f1.tile([P, heads], f32, tag="denr")
                        nc.vector.reciprocal(denr[:, :], den[:, :])
                        h1 = f1.tile([P, HC], f16, tag="h1")
                        nc.vector.tensor_tensor(
                            out=h1[:, :].rearrange("p (c h) -> p c h", h=heads),
                            in0=acc[:, :].rearrange("p (c h) -> p c h", h=heads),
                            in1=denr[:, :].unsqueeze(1)
                                .broadcast_to([P, hid, heads]),
                            op=OP.mult)
                        nc.vector.tensor_add(out=h1[:, :], in0=h1[:, :],
                                             in1=b1_sb[:, :])
                        mn = f1.tile([P, HC], f16, tag="mn")
                        nc.vector.tensor_scalar_min(mn[:, :], h1[:, :], 0.0)
                        nc.scalar.activation(mn[:, :], mn[:, :], AF.Exp)
                        nc.vector.tensor_scalar_max(h1[:, :], h1[:, :], 0.0)
                        nc.vector.scalar_tensor_tensor(
                            out=h1[:, :], in0=mn[:, :], scalar=-1.0,
                            in1=h1[:, :], op0=OP.add, op1=OP.add)
                        tp = f1.tile([P, HC], f16, tag="tp")
                        h2p = ps_h2.tile([P, R2], f32, tag="h2p", space="PSUM")
                        for ch in range(2):
                            tps = ps_tr.tile([P, P], f16, tag="tr", space="PSUM")
                            nc.tensor.transpose(
                                out=tps[:], in_=h1[:, ch * P : (ch + 1) * P],
                                identity=ident_sb[:])
                            nc.scalar.copy(out=tp[:, ch * P : (ch + 1) * P],
                                           in_=tps[:])
                            nc.tensor.matmul(
                                out=h2p[:, :], lhsT=tp[:, ch * P : (ch + 1) * P],
                                rhs=w2_sb[:, ch * R2 : (ch + 1) * R2],
                                start=(ch == 0), stop=(ch == 1))
                        st2 = f1.tile([P, R2], f16, tag="st2")
                        nc.scalar.copy(out=st2[:, :], in_=h2p[:, :])
                        nc.sync.dma_start(out=hx2in[b * P : (b + 1) * P, :],
                                          in_=st2[:, :])

                # ---------------- AllGather + copy out of Shared ----------------
                tc.strict_bb_all_engine_barrier()
                nc.gpsimd.collective_compute(
                    "AllGather", OP.bypass,
                    replica_groups=[list(range(NCORES))],
                    ins=[hx2in[:, :]], outs=[hx2[:, :]])
                tc.strict_bb_all_engine_barrier()

                with tc.tile_pool(name="cp2", bufs=3) as cph:
                    CB = 16
                    for j0 in range(0, NT, CB):
                        a = min(CB, NT - j0)
                        t = cph.tile([P, CB, R2], f16, tag="cp")
                        nc.sync.dma_start(
                            out=t[:, :a, :],
                            in_=hx2[j0 * P : (j0 + a) * P, :].rearrange(
                                "(a b) c -> b a c", b=P))
                        r0 = (j0 + 1) * P if j0 < NBLK_LO else (j0 + 2) * P
                        a1 = min(a, max(0, NBLK_LO - j0))
                        if a1:
                            nc.sync.dma_start(
                                out=hx2loc[r0 : r0 + a1 * P, :R2].rearrange(
                                    "(a b) c -> b a c", b=P),
                                in_=t[:, :a1, :])
                        if a1 < a:
                            r2_ = (j0 + a1 + 2) * P
                            nc.sync.dma_start(
                                out=hx2loc[r2_ : r2_ + (a - a1) * P, :R2]
                                    .rearrange("(a b) c -> b a c", b=P),
                                in_=t[:, a1:a, :])
                    ng2 = cph.tile([P, R2D], f16, tag="ng2")
                    nc.vector.memset(ng2[:, :cout], 0.0)
                    nc.vector.memset(ng2[:, cout : cout + 2], NEG_ES)
                    nc.vector.memset(ng2[:, cout + 2 : R2D], 0.0)
                    nc.sync.dma_start(out=hx2loc[0:P, :], in_=ng2[:, :])
                    nc.sync.dma_start(
                        out=hx2loc[SPLIT + P : SPLIT + 2 * P, :], in_=ng2[:, :])
                tc.strict_bb_all_engine_barrier()

                # ---------------- layer-2 edge phase ----------------
                blk_chunks = {}
                for ch in chunks:
                    blk_chunks.setdefault(ch[0], []).append(ch)
                with tc.tile_pool(name="e2", bufs=2) as e2, \
                     tc.tile_pool(name="f2", bufs=2) as f2:
                    for b in range(nblk):
                        K = int(K2lo[b]) + int(K2hi[b])
                        klo = int(K2lo[b])
                        hg = e2.tile([P, K, R2D], f16, tag="hg")
                        for (_, half, k0, kn, col0) in blk_chunks[b]:
                            base = k0 if half == 0 else klo + k0
                            src_ap = (hx2loc[0 : min(32768, NROW2), :]
                                      if half == 0
                                      else hx2loc[SPLIT + P : NROW2, :])
                            nc.gpsimd.dma_gather(
                                out_ap=hg[:, base : base + kn, :],
                                in_ap=src_ap,
                                idxs_ap=idx_sb[:, col0 : col0 + kn * 8],
                                num_idxs=kn * P, num_idxs_reg=kn * P,
                                elem_size=R2D)
                        edr2 = e2.tile([P, 2], f16, tag="edr2")
                        nc.gpsimd.indirect_dma_start(
                            out=edr2[:, :], out_offset=None,
                            in_=hx2loc[:, :],
                            in_offset=bass.IndirectOffsetOnAxis(
                                ap=own_sb[:, b : b + 1], axis=0),
                            element_offset=cout + 2)
                        ex = e2.tile([P, K, 2], f16, tag="ex2")
                        nc.vector.tensor_tensor(
                            out=ex[:, :, :], in0=hg[:, :, cout : cout + 2],
                            in1=edr2[:, :].unsqueeze(1)
                                .broadcast_to([P, K, 2]),
                            op=OP.add)
                        mx = e2.tile([P, K, 2], f16, tag="mx2")
                        nc.vector.tensor_scalar_max(mx[:, :, :], ex[:, :, :], 0.0)
                        nc.vector.tensor_scalar_min(ex[:, :, :], ex[:, :, :], 0.0)
                        nc.vector.scalar_tensor_tensor(
                            out=ex[:, :, :], in0=ex[:, :, :], scalar=NEG_SLOPE,
                            in1=mx[:, :, :], op0=OP.mult, op1=OP.add)
                        nc.scalar.activation(ex[:, :, :], ex[:, :, :], AF.Exp)
                        den = f2.tile([P, 2], f32, tag="den2")
                        nc.vector.tensor_reduce(
                            out=den[:, :], in_=ex[:, :, :].transpose([0, 2, 1]),
                            axis=AX.X, op=OP.add)
                        nc.vector.tensor_tensor(
                            out=hg[:, :, :R2].rearrange(
                                "p k (c h) -> p k c h", h=2),
                            in0=hg[:, :, :R2].rearrange(
                                "p k (c h) -> p k c h", h=2),
                            in1=ex[:, :, :].unsqueeze(2)
                                .broadcast_to([P, K, R2 // 2, 2]),
                            op=OP.mult)
                        accf = ps_acc.tile([P, HC], f32, tag="acc", space="PSUM")
                        acc = accf[:, :R2]
                        for k in range(K):
                            nc.tensor.matmul(out=acc[:, :], lhsT=ident_sb[:],
                                             rhs=hg[:, k, :R2],
                                             start=(k == 0), stop=(k == K - 1))
                        denr = f2.tile([P, 2], f32, tag="denr2")
                        nc.vector.reciprocal(denr[:, :], den[:, :])
                        o = f2.tile([P, cout], f32, tag="o")
                        nc.vector.tensor_tensor(
                            out=o[:, :].rearrange("p (c h) -> p c h", h=2),
                            in0=acc[:, :cout].rearrange("p (c h) -> p c h", h=2),
                            in1=denr[:, :].unsqueeze(1)
                                .broadcast_to([P, cout // 2, 2]),
                            op=OP.mult)
                        nc.vector.tensor_add(out=o[:, :], in0=o[:, :],
                                             in1=b2_sb[:, :])
                        nc.sync.dma_start(out=out[b * P : (b + 1) * P, :],
                                          in_=o[:, :])

    nc.compile()
    return nc


def host_prep(x, W1, a_src1, a_dst1, b1, W2, a_src2, a_dst2, b2, pl):
    n_nodes, cin = x.shape
    heads, hid = np.asarray(a_src1).shape
    HC = heads * hid
    cout = np.asarray(W2).shape[1]
    T = pl["T"]

    # W1ext: c-major h cols + es + ed
    W1 = np.asarray(W1, np.float32)
    W1h = W1.reshape(cin, heads, hid)
    W1cm = W1h.transpose(0, 2, 1).reshape(cin, HC)
    Wa1 = np.einsum("khc,hc->kh", W1h, np.asarray(a_src1, np.float32))
    Wb1 = np.einsum("khc,hc->kh", W1h, np.asarray(a_dst1, np.float32))
    Wx1 = np.concatenate([W1cm, Wa1, Wb1], 1).astype(np.float16)

    # x_neg: least-norm x with x@Wa1 == NEG_ES for every head
    G = Wa1.T @ Wa1
    x_neg = (Wa1 @ np.linalg.solve(G, np.full(heads, NEG_ES))).astype(np.float32)

    # per-core edge-order x operand
    xTf = np.asarray(x, np.float32).T.astype(np.float16)       # [cin, N]
    x_negf = x_neg.astype(np.float16)

    b1cm = np.asarray(b1, np.float32).reshape(heads, hid).T.reshape(HC)
    b1r = np.tile(b1cm[None, :], (P, 1)).astype(np.float16)

    W2 = np.asarray(W2, np.float32)
    rowperm = (np.arange(heads)[None, :] * hid
               + np.arange(hid)[:, None]).reshape(HC)
    Wa2 = (W2 * np.asarray(a_src2, np.float32)).sum(1)
    Wb2 = (W2 * np.asarray(a_dst2, np.float32)).sum(1)
    W2e = np.concatenate(
        [W2, Wa2[:, None], Wa2[:, None], Wb2[:, None], Wb2[:, None]], 1)
    W2e = W2e[rowperm].astype(np.float16)
    Wx2 = np.ascontiguousarray(np.concatenate([W2e[:P], W2e[P:]], axis=1))

    b2r = np.tile(np.asarray(b2, np.float32)[None, :], (P, 1)).astype(np.float32)

    common = {
        "Wx1": Wx1, "Wx2": Wx2, "b1r": b1r, "b2r": b2r,
        "identD": np.eye(P, dtype=np.float16),
    }
    in_maps = []
    for c in range(NCORES):
        sn = pl["srcnode"][c]                       # [P, T]
        # xE[:, t*P + p] = column for edge slot (p, t)
        cols = np.empty((cin, P, T), np.float16)
        valid = sn >= 0
        cols[:, valid] = xTf[:, sn[valid]]
        cols[:, sn == -1] = x_negf[:, None]
        cols[:, sn == -2] = 0.0
        xEc = cols.transpose(0, 2, 1).reshape(cin, T * P)
        m = dict(common)
        m["xE"] = np.ascontiguousarray(xEc)
        m["idx16"] = np.ascontiguousarray(pl["idx16"][c])
        m["ownrow"] = np.ascontiguousarray(pl["ownrow"][c])
        in_maps.append(m)
    return in_maps


def run_gat(x, edge_index, W1, a_src1, a_dst1, b1, W2, a_src2, a_dst2, b2,
            n_cores=NCORES, trace=False):
    n_nodes, cin = x.shape
    heads, hid = np.asarray(a_src1).shape
    cout = np.asarray(W2).shape[1]

    src = np.asarray(edge_index[0], np.int64)
    dst = np.asarray(edge_index[1], np.int64)
    pl = plan(src, dst, n_nodes, n_cores)
    in_maps = host_prep(x, W1, a_src1, a_dst1, b1, W2, a_src2, a_dst2, b2, pl)
    nc = build(pl, cin, heads, hid, cout)

    res = bass_utils.run_bass_kernel_spmd(
        nc, in_maps, core_ids=list(range(n_cores)), trace=trace)
    allout = np.concatenate([res.results[c]["out"] for c in range(n_cores)], 0)
    outp = allout[pl["pos_of_node"]]
    return outp.astype(np.float32), res


def kernel(**inputs):
    """Full-input GAT kernel: shards internally across 8 NeuronCores."""
    x = np.asarray(inputs["x"], np.float32)
    edge_index = np.asarray(inputs["edge_index"])
    outp, _ = run_gat(
        x, edge_index,
        inputs["W1"], inputs["a_src1"], inputs["a_dst1"], inputs["b1"],
        inputs["W2"], inputs["a_src2"], inputs["a_dst2"], inputs["b2"],
        n_cores=8, trace=bool(int(os.environ.get("GAT_TRACE", "0"))))
    return outp.astype(np.float32)
